# revision 1
# baseline (speedup 1.0000x reference)
"""2-layer LSTM (B=1024, T=256, I=64, H=128) + FC head on 8 NeuronCores.

Data-parallel: batch sharded 8 ways (128 rows/core), weights replicated.
On-chip orientation keeps state transposed (hT: [H partitions, B free]) so the
recurrence matmuls, activations and cell updates never need a transpose.
Gate order is repacked to (i, f, o, g) and the g-gate's tanh is computed as
2*sigmoid(2z)-1 so all four gate activations are one ACT instruction; the
affine fixup is folded into the fused DVE cell-update ops.
"""

import numpy as np

B, T, I, H = 1024, 256, 64, 128
NCORES = 8
BC = B // NCORES  # 128 batch rows per core
XCHUNK = 32  # timesteps per staged x DMA chunk
USE_BF16 = False  # matmul operands in bf16 (fp32 accumulate); elementwise stays fp32


def _mm_np_dtype():
    if USE_BF16:
        import ml_dtypes

        return ml_dtypes.bfloat16
    return np.float32


_cache = {}


def _build(has_b1, has_bfc, nsteps):
    import concourse.bacc as bacc
    import concourse.tile as tile
    import concourse.mybir as mybir

    f32 = mybir.dt.float32
    mdt = mybir.dt.bfloat16 if USE_BF16 else f32
    Alu = mybir.AluOpType
    Act = mybir.ActivationFunctionType

    nc = bacc.Bacc("TRN2", target_bir_lowering=False, debug=False)

    xt_d = nc.dram_tensor("xt", [I + 2, nsteps, BC], mdt, kind="ExternalInput")
    w0x_d = nc.dram_tensor("w0x", [4, I + 2, H], mdt, kind="ExternalInput")
    w0h_d = nc.dram_tensor("w0h", [4, H, H], mdt, kind="ExternalInput")
    w1x_d = nc.dram_tensor("w1x", [4, H, H], mdt, kind="ExternalInput")
    w1h_d = nc.dram_tensor("w1h", [4, H, H], mdt, kind="ExternalInput")
    wfc_d = nc.dram_tensor("wfc", [H, 1], mdt, kind="ExternalInput")
    b1_d = nc.dram_tensor("b1", [4, 1, H], mdt, kind="ExternalInput") if has_b1 else None
    bfc_d = nc.dram_tensor("bfc", [1, 1], mdt, kind="ExternalInput") if has_bfc else None
    out_d = nc.dram_tensor("out", [1, BC], f32, kind="ExternalOutput")

    with tile.TileContext(nc) as tc:
        with (
            tc.tile_pool(name="singles", bufs=1) as singles,
            tc.tile_pool(name="sg", bufs=3) as sgp,
            tc.tile_pool(name="tmp", bufs=4) as tmpp,
            tc.tile_pool(name="ps", bufs=3, space="PSUM") as psp,
            tc.tile_pool(name="psfc", bufs=1, space="PSUM") as psfc,
        ):
            xta = xt_d.ap()
            nchunk = (nsteps + XCHUNK - 1) // XCHUNK
            xts = []
            for j in range(nchunk):
                t0, t1 = j * XCHUNK, min((j + 1) * XCHUNK, nsteps)
                xt_t = singles.tile([I + 2, (t1 - t0) * BC], mdt, tag=f"xt{j}", name=f"xt{j}")
                nc.sync.dma_start(
                    out=xt_t[:], in_=xta[:, t0:t1, :].rearrange("p t b -> p (t b)")
                )
                xts.append(xt_t)

            def load_w(dram, k, q, tag):
                w = singles.tile([k, H], mdt, tag=f"{tag}{q}", name=f"{tag}{q}")
                nc.sync.dma_start(out=w[:], in_=dram.ap()[q])
                return w

            w0x = [load_w(w0x_d, I + 2, q, "w0x") for q in range(4)]
            w0h = [load_w(w0h_d, H, q, "w0h") for q in range(4)]
            w1x = [load_w(w1x_d, H, q, "w1x") for q in range(4)]
            w1h = [load_w(w1h_d, H, q, "w1h") for q in range(4)]
            wfc = singles.tile([H, 1], mdt, tag="wfc", name="wfc")
            nc.sync.dma_start(out=wfc[:], in_=wfc_d.ap())
            b1 = None
            ones = None
            if has_b1 or has_bfc:
                ones = singles.tile([1, BC], mdt, tag="ones", name="ones")
                nc.vector.memset(ones[:], 1.0)
            if has_b1:
                b1 = [load_w(b1_d, 1, q, "b1") for q in range(4)]
            bfc = None
            if has_bfc:
                bfc = singles.tile([1, 1], mdt, tag="bfc", name="bfc")
                nc.sync.dma_start(out=bfc[:], in_=bfc_d.ap())

            cs = []
            for layer in range(2):
                c = singles.tile([H, BC], f32, tag=f"c{layer}", name=f"c{layer}")
                nc.vector.memset(c[:], 0.0)
                cs.append(c)
            RING = 4
            rings = [
                [singles.tile([H, BC], mdt, tag=f"h{layer}r{s}", name=f"h{layer}r{s}") for s in range(RING)]
                for layer in range(2)
            ]

            def step(layer, t):
                wx, wh = (w0x, w0h) if layer == 0 else (w1x, w1h)
                ps = psp.tile([H, 4 * BC], f32, tag=f"ps{layer}", name=f"ps{layer}")
                for q in range(4):
                    sl = ps[:, q * BC : (q + 1) * BC]
                    if layer == 0:
                        j, r = t // XCHUNK, t % XCHUNK
                        rhs = xts[j][:, r * BC : (r + 1) * BC]
                    else:
                        rhs = rings[0][t % RING][:]
                    nc.tensor.matmul(sl, wx[q][:], rhs, start=True, stop=(t == 0 and not (has_b1 and layer == 1)))
                    if t > 0:
                        nc.tensor.matmul(
                            sl, wh[q][:], rings[layer][(t - 1) % RING][:],
                            start=False, stop=not (has_b1 and layer == 1),
                        )
                    if has_b1 and layer == 1:
                        nc.tensor.matmul(sl, b1[q][:], ones[:], start=False, stop=True)
                sg = sgp.tile([H, 4 * BC], f32, tag=f"sg{layer}", name=f"sg{layer}")
                nc.scalar.activation(sg[:], ps[:], Act.Sigmoid)
                t1_ = tmpp.tile([H, BC], f32, tag=f"t1_{layer}", name=f"t1_{layer}")
                # (sig_g - 0.5) * sig_i  == 0.5 * i * tanh(g_pre)
                nc.vector.scalar_tensor_tensor(
                    t1_[:], sg[:, 3 * BC : 4 * BC], 0.5, sg[:, 0:BC],
                    Alu.subtract, Alu.mult,
                )
                t2_ = tmpp.tile([H, BC], f32, tag=f"t2_{layer}", name=f"t2_{layer}")
                nc.vector.tensor_mul(t2_[:], sg[:, BC : 2 * BC], cs[layer][:])
                # c = 2*t1 + t2 = i*tanh(g_pre) + f*c
                nc.vector.scalar_tensor_tensor(
                    cs[layer][:], t1_[:], 2.0, t2_[:], Alu.mult, Alu.add
                )
                th = tmpp.tile([H, BC], f32, tag=f"th{layer}", name=f"th{layer}")
                nc.scalar.activation(th[:], cs[layer][:], Act.Tanh)
                h = rings[layer][t % RING]
                nc.vector.tensor_mul(h[:], sg[:, 2 * BC : 3 * BC], th[:])

            for t in range(nsteps):
                step(0, t)
                if t >= 1:
                    step(1, t - 1)
            step(1, nsteps - 1)

            pf = psfc.tile([1, BC], f32, tag="fc", name="fc")
            nc.tensor.matmul(
                pf[:], wfc[:], rings[1][(nsteps - 1) % RING][:],
                start=True, stop=not has_bfc,
            )
            if has_bfc:
                nc.tensor.matmul(pf[:], bfc[:], ones[:], start=False, stop=True)
            ot = singles.tile([1, BC], f32, tag="ot", name="ot")
            nc.vector.tensor_copy(ot[:], pf[:])
            nc.sync.dma_start(out=out_d.ap(), in_=ot[:])

    nc.compile()
    return nc


def _prep_weights(Wih, Whh, b, in_dim, fold_bias):
    """Repack [4H, in] PyTorch-gate-order (i,f,g,o) weights into per-gate
    lhsT tiles [in(+1), H] with gate order (i,f,o,g) and g scaled by 2."""
    order = [0, 1, 3, 2]  # i, f, o, g
    pad = 2 if fold_bias else 0
    wx = np.zeros((4, in_dim + pad, H), np.float32)
    wh = np.empty((4, H, H), np.float32)
    for qi, q in enumerate(order):
        scale = 2.0 if q == 2 else 1.0
        wx[qi, :in_dim] = (Wih[q * H : (q + 1) * H] * scale).T
        if fold_bias:
            wx[qi, in_dim] = b[q * H : (q + 1) * H] * scale
        wh[qi] = (Whh[q * H : (q + 1) * H] * scale).T
    return wx, wh


def kernel(x, Wih0, Whh0, b0, Wih1, Whh1, b1, Wfc, bfc, _nsteps=T):
    from concourse.bass_utils import run_bass_kernel_spmd

    x = np.asarray(x, np.float32)
    nsteps = _nsteps
    has_b1 = bool(np.any(np.asarray(b1)))
    has_bfc = bool(np.any(np.asarray(bfc)))

    w0x, w0h = _prep_weights(np.asarray(Wih0, np.float32), np.asarray(Whh0, np.float32),
                             np.asarray(b0, np.float32), I, True)
    w1x, w1h = _prep_weights(np.asarray(Wih1, np.float32), np.asarray(Whh1, np.float32),
                             np.asarray(b1, np.float32), H, False)
    wfc = np.ascontiguousarray(np.asarray(Wfc, np.float32).reshape(1, H).T)

    key = (has_b1, has_bfc, nsteps)
    if key not in _cache:
        _cache[key] = _build(has_b1, has_bfc, nsteps)
    nc = _cache[key]

    mnp = _mm_np_dtype()
    in_maps = []
    for c in range(NCORES):
        xc = x[c * BC : (c + 1) * BC, :nsteps]  # [BC, t, I]
        xt = np.zeros((I + 2, nsteps, BC), np.float32)
        xt[:I] = xc.transpose(2, 1, 0)
        xt[I] = 1.0
        m = {"xt": xt.astype(mnp), "w0x": w0x.astype(mnp), "w0h": w0h.astype(mnp),
             "w1x": w1x.astype(mnp), "w1h": w1h.astype(mnp), "wfc": wfc.astype(mnp)}
        if has_b1:
            border = [0, 1, 3, 2]
            bb = np.empty((4, 1, H), np.float32)
            for qi, q in enumerate(border):
                bb[qi, 0] = np.asarray(b1, np.float32)[q * H : (q + 1) * H] * (2.0 if q == 2 else 1.0)
            m["b1"] = bb.astype(mnp)
        if has_bfc:
            m["bfc"] = np.asarray(bfc, np.float32).reshape(1, 1).astype(mnp)
        in_maps.append(m)

    res = run_bass_kernel_spmd(nc, in_maps, list(range(NCORES)))
    globals()["LAST_RESULT"] = res
    globals()["LAST_RUN"] = (nc, in_maps)
    out = np.empty((B, 1), np.float32)
    for c in range(NCORES):
        out[c * BC : (c + 1) * BC, 0] = res.results[c]["out"][0]
    return out


def bench(iters=6):
    """Re-run the last compiled kernel, returning per-call wall seconds."""
    import time
    from concourse.bass_utils import run_bass_kernel_spmd

    nc, in_maps = globals()["LAST_RUN"]
    times = []
    for _ in range(iters):
        t0 = time.perf_counter()
        run_bass_kernel_spmd(nc, in_maps, list(range(NCORES)))
        times.append(time.perf_counter() - t0)
    return times



# revision 5
# speedup vs baseline: 1.2239x; 1.2239x over previous
"""2-layer LSTM (B=1024, T=256, I=64, H=128) + FC head on 8 NeuronCores.

Data-parallel: batch sharded 8 ways (128 rows/core), weights replicated.
On-chip orientation keeps state transposed (hT: [H partitions, B free]) so the
recurrence matmuls, activations and cell updates never need a transpose.
Gate order is repacked to (i, f, o, g) and the g-gate's tanh is computed as
2*sigmoid(2z)-1 so all four gate activations are one ACT instruction; the
affine fixup is folded into the fused DVE cell-update ops.
"""

import numpy as np

B, T, I, H = 1024, 256, 64, 128
NCORES = 8
BC = B // NCORES  # 128 batch rows per core
XCHUNK = 32  # timesteps per staged x DMA chunk
HALF = "float16"  # matmul operands + sigmoid outputs; cell state stays fp32


def _mm_np_dtype():
    if HALF == "bfloat16":
        import ml_dtypes

        return ml_dtypes.bfloat16
    if HALF == "float16":
        return np.float16
    return np.float32


_cache = {}


def _build(has_b1, has_bfc, nsteps):
    import concourse.bacc as bacc
    import concourse.tile as tile
    import concourse.mybir as mybir

    f32 = mybir.dt.float32
    mdt = getattr(mybir.dt, HALF)
    Alu = mybir.AluOpType
    Act = mybir.ActivationFunctionType

    nc = bacc.Bacc("TRN2", target_bir_lowering=False, debug=False)

    xt_d = nc.dram_tensor("xt", [I + 2, nsteps, BC], mdt, kind="ExternalInput")
    w0x_d = nc.dram_tensor("w0x", [4, I + 2, H], mdt, kind="ExternalInput")
    w0h_d = nc.dram_tensor("w0h", [4, H, H], mdt, kind="ExternalInput")
    w1x_d = nc.dram_tensor("w1x", [4, H, H], mdt, kind="ExternalInput")
    w1h_d = nc.dram_tensor("w1h", [4, H, H], mdt, kind="ExternalInput")
    wfc_d = nc.dram_tensor("wfc", [H, 1], mdt, kind="ExternalInput")
    b1_d = nc.dram_tensor("b1", [4, 1, H], mdt, kind="ExternalInput") if has_b1 else None
    bfc_d = nc.dram_tensor("bfc", [1, 1], mdt, kind="ExternalInput") if has_bfc else None
    out_d = nc.dram_tensor("out", [1, BC], f32, kind="ExternalOutput")

    with tile.TileContext(nc) as tc:
        with (
            tc.tile_pool(name="singles", bufs=1) as singles,
            tc.tile_pool(name="sg", bufs=3) as sgp,
            tc.tile_pool(name="tmp", bufs=4) as tmpp,
            tc.tile_pool(name="ps", bufs=3, space="PSUM") as psp,
            tc.tile_pool(name="psfc", bufs=1, space="PSUM") as psfc,
        ):
            xta = xt_d.ap()
            nchunk = (nsteps + XCHUNK - 1) // XCHUNK
            xts = []
            for j in range(nchunk):
                t0, t1 = j * XCHUNK, min((j + 1) * XCHUNK, nsteps)
                xt_t = singles.tile([I + 2, (t1 - t0) * BC], mdt, tag=f"xt{j}", name=f"xt{j}")
                nc.sync.dma_start(
                    out=xt_t[:], in_=xta[:, t0:t1, :].rearrange("p t b -> p (t b)")
                )
                xts.append(xt_t)

            def load_w(dram, k, q, tag):
                w = singles.tile([k, H], mdt, tag=f"{tag}{q}", name=f"{tag}{q}")
                nc.sync.dma_start(out=w[:], in_=dram.ap()[q])
                return w

            w0x = [load_w(w0x_d, I + 2, q, "w0x") for q in range(4)]
            w0h = [load_w(w0h_d, H, q, "w0h") for q in range(4)]
            w1x = [load_w(w1x_d, H, q, "w1x") for q in range(4)]
            w1h = [load_w(w1h_d, H, q, "w1h") for q in range(4)]
            wfc = singles.tile([H, 1], mdt, tag="wfc", name="wfc")
            nc.sync.dma_start(out=wfc[:], in_=wfc_d.ap())
            b1 = None
            ones = None
            if has_b1 or has_bfc:
                ones = singles.tile([1, BC], mdt, tag="ones", name="ones")
                nc.vector.memset(ones[:], 1.0)
            if has_b1:
                b1 = [load_w(b1_d, 1, q, "b1") for q in range(4)]
            bfc = None
            if has_bfc:
                bfc = singles.tile([1, 1], mdt, tag="bfc", name="bfc")
                nc.sync.dma_start(out=bfc[:], in_=bfc_d.ap())

            cs = []
            for layer in range(2):
                c = singles.tile([H, BC], f32, tag=f"c{layer}", name=f"c{layer}")
                nc.vector.memset(c[:], 0.0)
                cs.append(c)
            RING = 4
            rings = [
                [singles.tile([H, BC], mdt, tag=f"h{layer}r{s}", name=f"h{layer}r{s}") for s in range(RING)]
                for layer in range(2)
            ]

            def step(layer, t):
                wx, wh = (w0x, w0h) if layer == 0 else (w1x, w1h)
                ps = psp.tile([H, 4 * BC], f32, tag=f"ps{layer}", name=f"ps{layer}")
                for q in range(4):
                    sl = ps[:, q * BC : (q + 1) * BC]
                    if layer == 0:
                        j, r = t // XCHUNK, t % XCHUNK
                        rhs = xts[j][:, r * BC : (r + 1) * BC]
                    else:
                        rhs = rings[0][t % RING][:]
                    nc.tensor.matmul(sl, wx[q][:], rhs, start=True, stop=(t == 0 and not (has_b1 and layer == 1)))
                    if t > 0:
                        nc.tensor.matmul(
                            sl, wh[q][:], rings[layer][(t - 1) % RING][:],
                            start=False, stop=not (has_b1 and layer == 1),
                        )
                    if has_b1 and layer == 1:
                        nc.tensor.matmul(sl, b1[q][:], ones[:], start=False, stop=True)
                sg = sgp.tile([H, 4 * BC], mdt, tag=f"sg{layer}", name=f"sg{layer}")
                nc.scalar.activation(sg[:], ps[:], Act.Sigmoid)
                t1_ = tmpp.tile([H, BC], mdt, tag=f"t1_{layer}", name=f"t1_{layer}")
                # (sig_g - 0.5) * sig_i  == 0.5 * i * tanh(g_pre)
                nc.vector.scalar_tensor_tensor(
                    t1_[:], sg[:, 3 * BC : 4 * BC], 0.5, sg[:, 0:BC],
                    Alu.subtract, Alu.mult,
                )
                t2_ = tmpp.tile([H, BC], f32, tag=f"t2_{layer}", name=f"t2_{layer}")
                nc.vector.tensor_mul(t2_[:], sg[:, BC : 2 * BC], cs[layer][:])
                # c = 2*t1 + t2 = i*tanh(g_pre) + f*c
                nc.vector.scalar_tensor_tensor(
                    cs[layer][:], t1_[:], 2.0, t2_[:], Alu.mult, Alu.add
                )
                th = tmpp.tile([H, BC], mdt, tag=f"th{layer}", name=f"th{layer}")
                nc.scalar.activation(th[:], cs[layer][:], Act.Tanh)
                h = rings[layer][t % RING]
                nc.vector.tensor_mul(h[:], sg[:, 2 * BC : 3 * BC], th[:])

            for t in range(nsteps):
                step(0, t)
                if t >= 1:
                    step(1, t - 1)
            step(1, nsteps - 1)

            pf = psfc.tile([1, BC], f32, tag="fc", name="fc")
            nc.tensor.matmul(
                pf[:], wfc[:], rings[1][(nsteps - 1) % RING][:],
                start=True, stop=not has_bfc,
            )
            if has_bfc:
                nc.tensor.matmul(pf[:], bfc[:], ones[:], start=False, stop=True)
            ot = singles.tile([1, BC], f32, tag="ot", name="ot")
            nc.vector.tensor_copy(ot[:], pf[:])
            nc.sync.dma_start(out=out_d.ap(), in_=ot[:])

    nc.compile()
    return nc


def _prep_weights(Wih, Whh, b, in_dim, fold_bias):
    """Repack [4H, in] PyTorch-gate-order (i,f,g,o) weights into per-gate
    lhsT tiles [in(+1), H] with gate order (i,f,o,g) and g scaled by 2."""
    order = [0, 1, 3, 2]  # i, f, o, g
    pad = 2 if fold_bias else 0
    wx = np.zeros((4, in_dim + pad, H), np.float32)
    wh = np.empty((4, H, H), np.float32)
    for qi, q in enumerate(order):
        scale = 2.0 if q == 2 else 1.0
        wx[qi, :in_dim] = (Wih[q * H : (q + 1) * H] * scale).T
        if fold_bias:
            wx[qi, in_dim] = b[q * H : (q + 1) * H] * scale
        wh[qi] = (Whh[q * H : (q + 1) * H] * scale).T
    return wx, wh


def kernel(x, Wih0, Whh0, b0, Wih1, Whh1, b1, Wfc, bfc, _nsteps=T):
    from concourse.bass_utils import run_bass_kernel_spmd

    x = np.asarray(x, np.float32)
    nsteps = _nsteps
    has_b1 = bool(np.any(np.asarray(b1)))
    has_bfc = bool(np.any(np.asarray(bfc)))

    w0x, w0h = _prep_weights(np.asarray(Wih0, np.float32), np.asarray(Whh0, np.float32),
                             np.asarray(b0, np.float32), I, True)
    w1x, w1h = _prep_weights(np.asarray(Wih1, np.float32), np.asarray(Whh1, np.float32),
                             np.asarray(b1, np.float32), H, False)
    wfc = np.ascontiguousarray(np.asarray(Wfc, np.float32).reshape(1, H).T)

    key = (has_b1, has_bfc, nsteps)
    if key not in _cache:
        _cache[key] = _build(has_b1, has_bfc, nsteps)
    nc = _cache[key]

    mnp = _mm_np_dtype()
    in_maps = []
    for c in range(NCORES):
        xc = x[c * BC : (c + 1) * BC, :nsteps]  # [BC, t, I]
        xt = np.zeros((I + 2, nsteps, BC), np.float32)
        xt[:I] = xc.transpose(2, 1, 0)
        xt[I] = 1.0
        m = {"xt": xt.astype(mnp), "w0x": w0x.astype(mnp), "w0h": w0h.astype(mnp),
             "w1x": w1x.astype(mnp), "w1h": w1h.astype(mnp), "wfc": wfc.astype(mnp)}
        if has_b1:
            border = [0, 1, 3, 2]
            bb = np.empty((4, 1, H), np.float32)
            for qi, q in enumerate(border):
                bb[qi, 0] = np.asarray(b1, np.float32)[q * H : (q + 1) * H] * (2.0 if q == 2 else 1.0)
            m["b1"] = bb.astype(mnp)
        if has_bfc:
            m["bfc"] = np.asarray(bfc, np.float32).reshape(1, 1).astype(mnp)
        in_maps.append(m)

    res = run_bass_kernel_spmd(nc, in_maps, list(range(NCORES)))
    globals()["LAST_RESULT"] = res
    globals()["LAST_RUN"] = (nc, in_maps)
    out = np.empty((B, 1), np.float32)
    for c in range(NCORES):
        out[c * BC : (c + 1) * BC, 0] = res.results[c]["out"][0]
    return out


def bench(iters=6):
    """Re-run the last compiled kernel, returning per-call wall seconds."""
    import time
    from concourse.bass_utils import run_bass_kernel_spmd

    nc, in_maps = globals()["LAST_RUN"]
    times = []
    for _ in range(iters):
        t0 = time.perf_counter()
        run_bass_kernel_spmd(nc, in_maps, list(range(NCORES)))
        times.append(time.perf_counter() - t0)
    return times



# revision 13
# speedup vs baseline: 1.3042x; 1.0656x over previous
"""2-layer LSTM (B=1024, T=256, I=64, H=128) + FC head on 8 NeuronCores.

Data-parallel: batch sharded 8 ways (128 rows/core), weights replicated.
On-chip orientation keeps state transposed (hT: [H partitions, B free]) so the
recurrence matmuls, activations and cell updates never need a transpose.
Gate order is repacked to (i, f, o, g) and the g-gate's tanh is computed as
2*sigmoid(2z)-1 so all four gate activations are one ACT instruction; the
affine fixup is folded into the fused DVE cell-update ops.
"""

import numpy as np

B, T, I, H = 1024, 256, 64, 128
NCORES = 8
BC = B // NCORES  # 128 batch rows per core
XCHUNK = 32  # timesteps per staged x DMA chunk
CH = 2  # timesteps per x-projection chunk (double-buffered inside psum banks)
HALF = "float16"  # matmul operands + sigmoid outputs; cell state stays fp32


def _mm_np_dtype():
    if HALF == "bfloat16":
        import ml_dtypes

        return ml_dtypes.bfloat16
    if HALF == "float16":
        return np.float16
    return np.float32


_cache = {}


def _build(has_b1, has_bfc, nsteps):
    import concourse.bacc as bacc
    import concourse.tile as tile
    import concourse.mybir as mybir

    f32 = mybir.dt.float32
    mdt = getattr(mybir.dt, HALF)
    Alu = mybir.AluOpType
    Act = mybir.ActivationFunctionType

    nc = bacc.Bacc("TRN2", target_bir_lowering=False, debug=False)

    xt_d = nc.dram_tensor("xt", [I + 2, nsteps, BC], mdt, kind="ExternalInput")
    w0x_d = nc.dram_tensor("w0x", [4, I + 2, H], mdt, kind="ExternalInput")
    w0h_d = nc.dram_tensor("w0h", [4, H, H], mdt, kind="ExternalInput")
    w1x_d = nc.dram_tensor("w1x", [4, H, H], mdt, kind="ExternalInput")
    w1h_d = nc.dram_tensor("w1h", [4, H, H], mdt, kind="ExternalInput")
    wfc_d = nc.dram_tensor("wfc", [H, 1], mdt, kind="ExternalInput")
    b1_d = nc.dram_tensor("b1", [4, 1, H], mdt, kind="ExternalInput") if has_b1 else None
    bfc_d = nc.dram_tensor("bfc", [1, 1], mdt, kind="ExternalInput") if has_bfc else None
    out_d = nc.dram_tensor("out", [1, BC], f32, kind="ExternalOutput")

    with tile.TileContext(nc) as tc:
        with (
            tc.tile_pool(name="singles", bufs=1) as singles,
            tc.tile_pool(name="sg", bufs=3) as sgp,
            tc.tile_pool(name="tmp", bufs=4) as tmpp,
            tc.tile_pool(name="ps", bufs=1, space="PSUM") as psp,
        ):
            xta = xt_d.ap()
            nchunk = (nsteps + XCHUNK - 1) // XCHUNK
            xts = []
            for j in range(nchunk):
                t0, t1 = j * XCHUNK, min((j + 1) * XCHUNK, nsteps)
                xt_t = singles.tile([I + 2, (t1 - t0) * BC], mdt, tag=f"xt{j}", name=f"xt{j}")
                nc.sync.dma_start(
                    out=xt_t[:], in_=xta[:, t0:t1, :].rearrange("p t b -> p (t b)")
                )
                xts.append(xt_t)

            def load_w(dram, k, q, tag):
                w = singles.tile([k, H], mdt, tag=f"{tag}{q}", name=f"{tag}{q}")
                nc.sync.dma_start(out=w[:], in_=dram.ap()[q])
                return w

            w0x = [load_w(w0x_d, I + 2, q, "w0x") for q in range(4)]
            w0h = [load_w(w0h_d, H, q, "w0h") for q in range(4)]
            w1x = [load_w(w1x_d, H, q, "w1x") for q in range(4)]
            w1h = [load_w(w1h_d, H, q, "w1h") for q in range(4)]
            wfc = singles.tile([H, 1], mdt, tag="wfc", name="wfc")
            nc.sync.dma_start(out=wfc[:], in_=wfc_d.ap())
            b1 = None
            ones = None
            if has_b1 or has_bfc:
                ones = singles.tile([1, BC], mdt, tag="ones", name="ones")
                nc.vector.memset(ones[:], 1.0)
            if has_b1:
                b1 = [load_w(b1_d, 1, q, "b1") for q in range(4)]
            bfc = None
            if has_bfc:
                bfc = singles.tile([1, 1], mdt, tag="bfc", name="bfc")
                nc.sync.dma_start(out=bfc[:], in_=bfc_d.ap())

            cs = []
            for layer in range(2):
                c = singles.tile([H, BC], f32, tag=f"c{layer}", name=f"c{layer}")
                nc.vector.memset(c[:], 0.0)
                cs.append(c)
            RING = 4
            rings = [
                singles.tile([H, RING * BC], mdt, tag=f"h{layer}", name=f"h{layer}")
                for layer in range(2)
            ]

            def hslot(layer, t):
                s = t % RING
                return rings[layer][:, s * BC : (s + 1) * BC]

            # PSUM layout: one full 2KB bank per (layer, gate); only the first
            # BC columns are used. No step rotation: step t+1's x-projection
            # (start=True, lazily zeroing the bank) is WAR-serialized after
            # sigma(t)'s read of the same bytes, so each bank's accumulation
            # groups stay strictly sequential: x(start) -> h(stop) -> sigma
            # read -> x(start) -> ...
            pst = [
                psp.tile([H, 4, 512], f32, tag=f"ps{layer}", name=f"ps{layer}")
                for layer in range(2)
            ]

            def gate_out(layer, q):
                return pst[layer][:, q : q + 1, 0:BC].rearrange("p a b -> p (a b)")

            def sig_in(layer):
                return pst[layer][:, :, 0:BC]

            def x_rhs(t):
                j, r = t // XCHUNK, t % XCHUNK
                return xts[j][:, r * BC : (r + 1) * BC]

            def emit_l0_x(t):
                for q in range(4):
                    nc.tensor.matmul(
                        gate_out(0, q), w0x[q][:], x_rhs(t),
                        start=True, stop=(t == 0),
                    )

            def emit_l0_h(t):
                for q in range(4):
                    nc.tensor.matmul(
                        gate_out(0, q), w0h[q][:], hslot(0, t - 1),
                        start=False, stop=True,
                    )

            def emit_l1_x(t):
                for q in range(4):
                    nc.tensor.matmul(
                        gate_out(1, q), w1x[q][:], hslot(0, t),
                        start=True, stop=(t == 0 and not has_b1),
                    )
                    if has_b1:
                        nc.tensor.matmul(
                            gate_out(1, q), b1[q][:], ones[:],
                            start=False, stop=(t == 0),
                        )

            def emit_l1_h(t):
                for q in range(4):
                    nc.tensor.matmul(
                        gate_out(1, q), w1h[q][:], hslot(1, t - 1),
                        start=False, stop=True,
                    )

            def act_dve(layer, t):
                sg = sgp.tile([H, 4 * BC], mdt, tag=f"sg{layer}", name=f"sg{layer}")
                nc.scalar.activation(
                    sg[:].rearrange("p (g b) -> p g b", g=4),
                    sig_in(layer), Act.Sigmoid,
                )
                t1_ = tmpp.tile([H, BC], mdt, tag=f"t1_{layer}", name=f"t1_{layer}")
                # (sig_g - 0.5) * sig_i  == 0.5 * i * tanh(g_pre)
                nc.vector.scalar_tensor_tensor(
                    t1_[:], sg[:, 3 * BC : 4 * BC], 0.5, sg[:, 0:BC],
                    Alu.subtract, Alu.mult,
                )
                # f * c on the otherwise-idle gpsimd engine, in parallel with
                # t1 on the vector engine (both feed the c update)
                t2_ = tmpp.tile([H, BC], f32, tag=f"t2_{layer}", name=f"t2_{layer}")
                nc.gpsimd.tensor_mul(t2_[:], sg[:, BC : 2 * BC], cs[layer][:])
                # c = 2*t1 + t2 = i*tanh(g_pre) + f*c
                nc.vector.scalar_tensor_tensor(
                    cs[layer][:], t1_[:], 2.0, t2_[:], Alu.mult, Alu.add
                )
                th = tmpp.tile([H, BC], mdt, tag=f"th{layer}", name=f"th{layer}")
                nc.scalar.activation(th[:], cs[layer][:], Act.Tanh)
                nc.vector.tensor_mul(hslot(layer, t), sg[:, 2 * BC : 3 * BC], th[:])

            emit_l0_x(0)
            for t in range(nsteps):
                if t >= 1:
                    emit_l0_h(t)
                    emit_l1_x(t - 1)
                    if t - 1 >= 1:
                        emit_l1_h(t - 1)
                act_dve(0, t)
                if t >= 1:
                    act_dve(1, t - 1)
                # next step's x-projections, emitted after sigma(t)'s read so
                # the bank WAR serializes them behind it (off critical path)
                if t + 1 < nsteps:
                    emit_l0_x(t + 1)
            emit_l1_x(nsteps - 1)
            if nsteps - 1 >= 1:
                emit_l1_h(nsteps - 1)
            act_dve(1, nsteps - 1)

            # FC head reuses a closed L0 psum bank region.
            pf = pst[0][0:1, 0:1, 0:BC].rearrange("p a b -> p (a b)")
            nc.tensor.matmul(
                pf, wfc[:], hslot(1, nsteps - 1),
                start=True, stop=not has_bfc,
            )
            if has_bfc:
                nc.tensor.matmul(pf, bfc[:], ones[:], start=False, stop=True)
            ot = singles.tile([1, BC], f32, tag="ot", name="ot")
            nc.vector.tensor_copy(ot[:], pf)
            nc.sync.dma_start(out=out_d.ap(), in_=ot[:])

    nc.compile()
    return nc


def _prep_weights(Wih, Whh, b, in_dim, fold_bias):
    """Repack [4H, in] PyTorch-gate-order (i,f,g,o) weights into per-gate
    lhsT tiles [in(+1), H] with gate order (i,f,o,g) and g scaled by 2."""
    order = [0, 1, 3, 2]  # i, f, o, g
    pad = 2 if fold_bias else 0
    wx = np.zeros((4, in_dim + pad, H), np.float32)
    wh = np.empty((4, H, H), np.float32)
    for qi, q in enumerate(order):
        scale = 2.0 if q == 2 else 1.0
        wx[qi, :in_dim] = (Wih[q * H : (q + 1) * H] * scale).T
        if fold_bias:
            wx[qi, in_dim] = b[q * H : (q + 1) * H] * scale
        wh[qi] = (Whh[q * H : (q + 1) * H] * scale).T
    return wx, wh


def kernel(x, Wih0, Whh0, b0, Wih1, Whh1, b1, Wfc, bfc, _nsteps=T):
    from concourse.bass_utils import run_bass_kernel_spmd

    x = np.asarray(x, np.float32)
    nsteps = _nsteps
    has_b1 = bool(np.any(np.asarray(b1)))
    has_bfc = bool(np.any(np.asarray(bfc)))

    w0x, w0h = _prep_weights(np.asarray(Wih0, np.float32), np.asarray(Whh0, np.float32),
                             np.asarray(b0, np.float32), I, True)
    w1x, w1h = _prep_weights(np.asarray(Wih1, np.float32), np.asarray(Whh1, np.float32),
                             np.asarray(b1, np.float32), H, False)
    wfc = np.ascontiguousarray(np.asarray(Wfc, np.float32).reshape(1, H).T)

    key = (has_b1, has_bfc, nsteps)
    if key not in _cache:
        _cache[key] = _build(has_b1, has_bfc, nsteps)
    nc = _cache[key]

    mnp = _mm_np_dtype()
    in_maps = []
    for c in range(NCORES):
        xc = x[c * BC : (c + 1) * BC, :nsteps]  # [BC, t, I]
        xt = np.zeros((I + 2, nsteps, BC), np.float32)
        xt[:I] = xc.transpose(2, 1, 0)
        xt[I] = 1.0
        m = {"xt": xt.astype(mnp), "w0x": w0x.astype(mnp), "w0h": w0h.astype(mnp),
             "w1x": w1x.astype(mnp), "w1h": w1h.astype(mnp), "wfc": wfc.astype(mnp)}
        if has_b1:
            border = [0, 1, 3, 2]
            bb = np.empty((4, 1, H), np.float32)
            for qi, q in enumerate(border):
                bb[qi, 0] = np.asarray(b1, np.float32)[q * H : (q + 1) * H] * (2.0 if q == 2 else 1.0)
            m["b1"] = bb.astype(mnp)
        if has_bfc:
            m["bfc"] = np.asarray(bfc, np.float32).reshape(1, 1).astype(mnp)
        in_maps.append(m)

    res = run_bass_kernel_spmd(nc, in_maps, list(range(NCORES)))
    globals()["LAST_RESULT"] = res
    globals()["LAST_RUN"] = (nc, in_maps)
    out = np.empty((B, 1), np.float32)
    for c in range(NCORES):
        out[c * BC : (c + 1) * BC, 0] = res.results[c]["out"][0]
    return out


def bench(iters=6):
    """Re-run the last compiled kernel, returning per-call wall seconds."""
    import time
    from concourse.bass_utils import run_bass_kernel_spmd

    nc, in_maps = globals()["LAST_RUN"]
    times = []
    for _ in range(iters):
        t0 = time.perf_counter()
        run_bass_kernel_spmd(nc, in_maps, list(range(NCORES)))
        times.append(time.perf_counter() - t0)
    return times



# revision 15
# speedup vs baseline: 1.3909x; 1.0665x over previous
"""2-layer LSTM (B=1024, T=256, I=64, H=128) + FC head on 8 NeuronCores.

Data-parallel: batch sharded 8 ways (128 rows/core), weights replicated.
On-chip orientation keeps state transposed (hT: [H partitions, B free]) so the
recurrence matmuls, activations and cell updates never need a transpose.
Gate order is repacked to (i, f, o, g) and the g-gate's tanh is computed as
2*sigmoid(2z)-1 so all four gate activations are one ACT instruction; the
affine fixup is folded into the fused DVE cell-update ops.
"""

import numpy as np

B, T, I, H = 1024, 256, 64, 128
NCORES = 8
BC = B // NCORES  # 128 batch rows per core
XCHUNK = 32  # timesteps per staged x DMA chunk
CH = 2  # timesteps per x-projection chunk (double-buffered inside psum banks)
HALF = "float16"  # matmul operands + sigmoid outputs; cell state stays fp32


def _mm_np_dtype():
    if HALF == "bfloat16":
        import ml_dtypes

        return ml_dtypes.bfloat16
    if HALF == "float16":
        return np.float16
    return np.float32


_cache = {}


def _build(has_b1, has_bfc, nsteps):
    import concourse.bacc as bacc
    import concourse.tile as tile
    import concourse.mybir as mybir

    f32 = mybir.dt.float32
    mdt = getattr(mybir.dt, HALF)
    Alu = mybir.AluOpType
    Act = mybir.ActivationFunctionType

    nc = bacc.Bacc("TRN2", target_bir_lowering=False, debug=False)

    xt_d = nc.dram_tensor("xt", [I + 2, nsteps, BC], mdt, kind="ExternalInput")
    w0x_d = nc.dram_tensor("w0x", [4, I + 2, H], mdt, kind="ExternalInput")
    w0h_d = nc.dram_tensor("w0h", [4, H, H], mdt, kind="ExternalInput")
    w1x_d = nc.dram_tensor("w1x", [4, H, H], mdt, kind="ExternalInput")
    w1h_d = nc.dram_tensor("w1h", [4, H, H], mdt, kind="ExternalInput")
    wfc_d = nc.dram_tensor("wfc", [H, 1], mdt, kind="ExternalInput")
    b1_d = nc.dram_tensor("b1", [4, 1, H], mdt, kind="ExternalInput") if has_b1 else None
    bfc_d = nc.dram_tensor("bfc", [1, 1], mdt, kind="ExternalInput") if has_bfc else None
    out_d = nc.dram_tensor("out", [1, BC], f32, kind="ExternalOutput")

    with tile.TileContext(nc) as tc:
        with (
            tc.tile_pool(name="singles", bufs=1) as singles,
            tc.tile_pool(name="sg", bufs=3) as sgp,
            tc.tile_pool(name="tmp", bufs=4) as tmpp,
            tc.tile_pool(name="ps", bufs=1, space="PSUM") as psp,
        ):
            xta = xt_d.ap()
            nchunk = (nsteps + XCHUNK - 1) // XCHUNK
            xts = []
            for j in range(nchunk):
                t0, t1 = j * XCHUNK, min((j + 1) * XCHUNK, nsteps)
                xt_t = singles.tile([I + 2, (t1 - t0) * BC], mdt, tag=f"xt{j}", name=f"xt{j}")
                nc.sync.dma_start(
                    out=xt_t[:], in_=xta[:, t0:t1, :].rearrange("p t b -> p (t b)")
                )
                xts.append(xt_t)

            def load_w(dram, k, q, tag):
                w = singles.tile([k, H], mdt, tag=f"{tag}{q}", name=f"{tag}{q}")
                nc.sync.dma_start(out=w[:], in_=dram.ap()[q])
                return w

            w0x = [load_w(w0x_d, I + 2, q, "w0x") for q in range(4)]
            w0h = [load_w(w0h_d, H, q, "w0h") for q in range(4)]
            w1x = [load_w(w1x_d, H, q, "w1x") for q in range(4)]
            w1h = [load_w(w1h_d, H, q, "w1h") for q in range(4)]
            wfc = singles.tile([H, 1], mdt, tag="wfc", name="wfc")
            nc.sync.dma_start(out=wfc[:], in_=wfc_d.ap())
            b1 = None
            ones = None
            if has_b1 or has_bfc:
                ones = singles.tile([1, BC], mdt, tag="ones", name="ones")
                nc.vector.memset(ones[:], 1.0)
            if has_b1:
                b1 = [load_w(b1_d, 1, q, "b1") for q in range(4)]
            bfc = None
            if has_bfc:
                bfc = singles.tile([1, 1], mdt, tag="bfc", name="bfc")
                nc.sync.dma_start(out=bfc[:], in_=bfc_d.ap())

            # half-scale cell state: cs holds c/2 (fp16 so DVE 2x applies)
            cs = []
            for layer in range(2):
                c = singles.tile([H, BC], mdt, tag=f"c{layer}", name=f"c{layer}")
                nc.vector.memset(c[:], 0.0)
                cs.append(c)
            RING = 4
            rings = [
                singles.tile([H, RING * BC], mdt, tag=f"h{layer}", name=f"h{layer}")
                for layer in range(2)
            ]

            def hslot(layer, t):
                s = t % RING
                return rings[layer][:, s * BC : (s + 1) * BC]

            # PSUM layout: one full 2KB bank per (layer, gate); only the first
            # BC columns are used. No step rotation: step t+1's x-projection
            # (start=True, lazily zeroing the bank) is WAR-serialized after
            # sigma(t)'s read of the same bytes, so each bank's accumulation
            # groups stay strictly sequential: x(start) -> h(stop) -> sigma
            # read -> x(start) -> ...
            pst = [
                psp.tile([H, 4, 512], f32, tag=f"ps{layer}", name=f"ps{layer}")
                for layer in range(2)
            ]

            def gate_out(layer, q):
                return pst[layer][:, q : q + 1, 0:BC].rearrange("p a b -> p (a b)")

            def sig_in(layer):
                return pst[layer][:, :, 0:BC]

            def x_rhs(t):
                j, r = t // XCHUNK, t % XCHUNK
                return xts[j][:, r * BC : (r + 1) * BC]

            def emit_l0_x(t):
                for q in range(4):
                    nc.tensor.matmul(
                        gate_out(0, q), w0x[q][:], x_rhs(t),
                        start=True, stop=(t == 0),
                    )

            def emit_l0_h(t):
                for q in range(4):
                    nc.tensor.matmul(
                        gate_out(0, q), w0h[q][:], hslot(0, t - 1),
                        start=False, stop=True,
                    )

            def emit_l1_x(t):
                for q in range(4):
                    nc.tensor.matmul(
                        gate_out(1, q), w1x[q][:], hslot(0, t),
                        start=True, stop=(t == 0 and not has_b1),
                    )
                    if has_b1:
                        nc.tensor.matmul(
                            gate_out(1, q), b1[q][:], ones[:],
                            start=False, stop=(t == 0),
                        )

            def emit_l1_h(t):
                for q in range(4):
                    nc.tensor.matmul(
                        gate_out(1, q), w1h[q][:], hslot(1, t - 1),
                        start=False, stop=True,
                    )

            def act_dve(layer, t):
                sg = sgp.tile([H, 4 * BC], mdt, tag=f"sg{layer}", name=f"sg{layer}")
                nc.scalar.activation(
                    sg[:].rearrange("p (g b) -> p g b", g=4),
                    sig_in(layer), Act.Sigmoid,
                )
                t1_ = tmpp.tile([H, BC], mdt, tag=f"t1_{layer}", name=f"t1_{layer}")
                # (sig_g - 0.5) * sig_i  == 0.5 * i * tanh(g_pre)
                nc.vector.scalar_tensor_tensor(
                    t1_[:], sg[:, 3 * BC : 4 * BC], 0.5, sg[:, 0:BC],
                    Alu.subtract, Alu.mult,
                )
                t2_ = tmpp.tile([H, BC], mdt, tag=f"t2_{layer}", name=f"t2_{layer}")
                nc.vector.tensor_mul(t2_[:], sg[:, BC : 2 * BC], cs[layer][:])
                # c' = t1 + t2 = (i*tanh(g_pre) + f*c)/2  (half-scale state)
                nc.vector.tensor_add(cs[layer][:], t1_[:], t2_[:])
                th = tmpp.tile([H, BC], mdt, tag=f"th{layer}", name=f"th{layer}")
                # tanh(c) = tanh(2*c')
                nc.scalar.activation(th[:], cs[layer][:], Act.Tanh, scale=2.0)
                nc.vector.tensor_mul(hslot(layer, t), sg[:, 2 * BC : 3 * BC], th[:])

            emit_l0_x(0)
            for t in range(nsteps):
                if t >= 1:
                    emit_l0_h(t)
                    emit_l1_x(t - 1)
                    if t - 1 >= 1:
                        emit_l1_h(t - 1)
                act_dve(0, t)
                if t >= 1:
                    act_dve(1, t - 1)
                # next step's x-projections, emitted after sigma(t)'s read so
                # the bank WAR serializes them behind it (off critical path)
                if t + 1 < nsteps:
                    emit_l0_x(t + 1)
            emit_l1_x(nsteps - 1)
            if nsteps - 1 >= 1:
                emit_l1_h(nsteps - 1)
            act_dve(1, nsteps - 1)

            # FC head reuses a closed L0 psum bank region.
            pf = pst[0][0:1, 0:1, 0:BC].rearrange("p a b -> p (a b)")
            nc.tensor.matmul(
                pf, wfc[:], hslot(1, nsteps - 1),
                start=True, stop=not has_bfc,
            )
            if has_bfc:
                nc.tensor.matmul(pf, bfc[:], ones[:], start=False, stop=True)
            ot = singles.tile([1, BC], f32, tag="ot", name="ot")
            nc.vector.tensor_copy(ot[:], pf)
            nc.sync.dma_start(out=out_d.ap(), in_=ot[:])

    nc.compile()
    return nc


def _prep_weights(Wih, Whh, b, in_dim, fold_bias):
    """Repack [4H, in] PyTorch-gate-order (i,f,g,o) weights into per-gate
    lhsT tiles [in(+1), H] with gate order (i,f,o,g) and g scaled by 2."""
    order = [0, 1, 3, 2]  # i, f, o, g
    pad = 2 if fold_bias else 0
    wx = np.zeros((4, in_dim + pad, H), np.float32)
    wh = np.empty((4, H, H), np.float32)
    for qi, q in enumerate(order):
        scale = 2.0 if q == 2 else 1.0
        wx[qi, :in_dim] = (Wih[q * H : (q + 1) * H] * scale).T
        if fold_bias:
            wx[qi, in_dim] = b[q * H : (q + 1) * H] * scale
        wh[qi] = (Whh[q * H : (q + 1) * H] * scale).T
    return wx, wh


def kernel(x, Wih0, Whh0, b0, Wih1, Whh1, b1, Wfc, bfc, _nsteps=T):
    from concourse.bass_utils import run_bass_kernel_spmd

    x = np.asarray(x, np.float32)
    nsteps = _nsteps
    has_b1 = bool(np.any(np.asarray(b1)))
    has_bfc = bool(np.any(np.asarray(bfc)))

    w0x, w0h = _prep_weights(np.asarray(Wih0, np.float32), np.asarray(Whh0, np.float32),
                             np.asarray(b0, np.float32), I, True)
    w1x, w1h = _prep_weights(np.asarray(Wih1, np.float32), np.asarray(Whh1, np.float32),
                             np.asarray(b1, np.float32), H, False)
    wfc = np.ascontiguousarray(np.asarray(Wfc, np.float32).reshape(1, H).T)

    key = (has_b1, has_bfc, nsteps)
    if key not in _cache:
        _cache[key] = _build(has_b1, has_bfc, nsteps)
    nc = _cache[key]

    mnp = _mm_np_dtype()
    in_maps = []
    for c in range(NCORES):
        xc = x[c * BC : (c + 1) * BC, :nsteps]  # [BC, t, I]
        xt = np.zeros((I + 2, nsteps, BC), np.float32)
        xt[:I] = xc.transpose(2, 1, 0)
        xt[I] = 1.0
        m = {"xt": xt.astype(mnp), "w0x": w0x.astype(mnp), "w0h": w0h.astype(mnp),
             "w1x": w1x.astype(mnp), "w1h": w1h.astype(mnp), "wfc": wfc.astype(mnp)}
        if has_b1:
            border = [0, 1, 3, 2]
            bb = np.empty((4, 1, H), np.float32)
            for qi, q in enumerate(border):
                bb[qi, 0] = np.asarray(b1, np.float32)[q * H : (q + 1) * H] * (2.0 if q == 2 else 1.0)
            m["b1"] = bb.astype(mnp)
        if has_bfc:
            m["bfc"] = np.asarray(bfc, np.float32).reshape(1, 1).astype(mnp)
        in_maps.append(m)

    res = run_bass_kernel_spmd(nc, in_maps, list(range(NCORES)))
    globals()["LAST_RESULT"] = res
    globals()["LAST_RUN"] = (nc, in_maps)
    out = np.empty((B, 1), np.float32)
    for c in range(NCORES):
        out[c * BC : (c + 1) * BC, 0] = res.results[c]["out"][0]
    return out


def bench(iters=6):
    """Re-run the last compiled kernel, returning per-call wall seconds."""
    import time
    from concourse.bass_utils import run_bass_kernel_spmd

    nc, in_maps = globals()["LAST_RUN"]
    times = []
    for _ in range(iters):
        t0 = time.perf_counter()
        run_bass_kernel_spmd(nc, in_maps, list(range(NCORES)))
        times.append(time.perf_counter() - t0)
    return times

